# revision 44
# baseline (speedup 1.0000x reference)
"""Branched attention processor (SDXL-like) on 8 Trainium2 NeuronCores.

Sharding: 2-way data-parallel over the half-batch dim x 4-way tensor-parallel
over heads (5 heads = 320 features per core). Each core computes a partial
out^T = Wo[:, c_slice] @ merged[c_slice, :]; the host sums the 4 partials per
half-batch and adds the bias.

v3: bf16 numerics end to end (fp8 injects >=4% per-weight noise which the
softmax passes through at full strength -- measured, not theoretical).
  - xt / weights / q / k / v / pt / merged all bf16; PSUM accumulate f32.
  - k stored per head zero-padded to 128 rows (full-rate contraction);
    face-branch gate folded into k/v data so exp needs no per-partition
    scale and both branches share one activation.
  - exp runs on Act only (nothing else can do exp); everything else is kept
    off Act during attention so its 16 x 1.04us per round sets the pace.
  - Tail: PSUM accumulators drained to SBUF bf16 (DVE), sums row DMA'd to
    partition 0, one reciprocal (DVE), gpsimd partition_broadcast (Pool),
    bf16 normalize mults (Pool) + merge add (DVE); software-pipelined into
    the next round's slots. AV matmuls are emitted with a 4-slot lag so the
    in-order PE queue never head-blocks on the previous round's PSUM WAR.
  - Wo for sq-half 0 is interleaved into attention rounds 5..9 (mT cols
    0:1024 are complete after round 4); only half 1 remains serial.
"""

import os as _os

import numpy as np

import concourse.bass as bass
import concourse.tile as tile
import concourse.mybir as mybir
from concourse import bacc
from concourse.bass_utils import run_bass_kernel_spmd

# Problem shapes (hardcoded per contract)
B2, S, C = 4, 1024, 1280
B = B2 // 2           # 2 half-batches
H = 20                # heads
D = C // H            # 64
G = 4                 # head groups (tensor-parallel)
HG = H // G           # 5 heads per core
OS = HG * D           # 320 features per core
SQ = 2 * S            # 2048 queries per half-batch
P = 128
NCHUNK = C // P       # 10 c-chunks of 128

F32 = mybir.dt.float32
BF16 = mybir.dt.bfloat16
U16 = mybir.dt.uint16
EXP = mybir.ActivationFunctionType.Exp
MULT = mybir.AluOpType.mult


def build_nc(iters: int = 1):
    nc = bacc.Bacc("TRN2", target_bir_lowering=False, debug=False, num_devices=8)

    xt16 = nc.dram_tensor("xt16", [C, 2 * SQ], BF16, kind="ExternalInput")
    wq16 = nc.dram_tensor("wq16", [C, OS], BF16, kind="ExternalInput")
    wk16 = nc.dram_tensor("wk16", [C, OS], BF16, kind="ExternalInput")
    wv16 = nc.dram_tensor("wv16", [C, OS], BF16, kind="ExternalInput")
    wo16 = nc.dram_tensor("wo16", [OS, C], BF16, kind="ExternalInput")
    gb16 = nc.dram_tensor("gb16", [P, S], BF16, kind="ExternalInput")
    gv = nc.dram_tensor("gv", [P, 8], F32, kind="ExternalInput")
    outp = nc.dram_tensor("outp", [C, SQ], BF16, kind="ExternalOutput")

    abl_skip_attn = bool(_os.environ.get("KSKIP_ATTN"))
    abl_skip_wo = bool(_os.environ.get("KSKIP_WO"))
    abl_no_tail = bool(_os.environ.get("KATT_NOTAIL"))

    with tile.TileContext(nc) as tc, nc.allow_low_precision(
            reason="bf16 kernel by design"):
        with (
            tc.tile_pool(name="persist", bufs=1) as persist,
            tc.tile_pool(name="work", bufs=2) as work,
            tc.tile_pool(name="ptp", bufs=6) as ptp,
            tc.tile_pool(name="outsb", bufs=2) as outsb,
            tc.tile_pool(name="ps_l", bufs=2, space="PSUM") as ps_l,
            tc.tile_pool(name="ps_av", bufs=4, space="PSUM") as ps_av,
        ):
            # ---- persistent tiles ----
            # q^T per unit (pairs of heads); unit 2 rows 64:128 unused
            qT = [persist.tile([P, SQ], BF16, tag=f"qT{i}", name=f"qT{i}")
                  for i in range(3)]
            # k^T per head, zero-padded to 128 rows: head h lives at rows
            # (h%2)*64, the other 64 rows stay zero (full 128-contraction).
            # cols 0:1024 noise, 1024:2048 ref (gate folded in).
            kTz = [persist.tile([P, SQ], BF16, tag=f"kTz{h}", name=f"kTz{h}")
                   for h in range(HG)]
            # v_aug per (branch, sk tile): [128, 5, 65]: 64 v cols + ones
            va16 = [[persist.tile([P, HG * (D + 1)], BF16, tag=f"va{br}_{t}",
                                  name=f"va{br}_{t}")
                     for t in range(8)] for br in range(2)]
            # merged^T bf16 per unit; unit 2 rows 64:128 zeroed
            mT = [persist.tile([P, SQ], BF16, tag=f"mT{i}", name=f"mT{i}")
                  for i in range(3)]
            wq_sb = persist.tile([P, NCHUNK * OS], BF16, tag="wq")
            wk_sb = persist.tile([P, NCHUNK * OS], BF16, tag="wk")
            wv_sb = persist.tile([P, NCHUNK * OS], BF16, tag="wv")
            wo_sb = [persist.tile([P, C], BF16, tag=f"wo{i}", name=f"wo{i}")
                     for i in range(3)]
            gb_sb = persist.tile([P, S], BF16, tag="gb")
            gv_sb = persist.tile([P, 8], F32, tag="gvs")

            for w_sb, w_dram in ((wq_sb, wq16), (wk_sb, wk16), (wv_sb, wv16)):
                nc.sync.dma_start(
                    w_sb[:].rearrange("p (g m) -> p g m", m=OS),
                    w_dram[0:C, :].rearrange("(g p) m -> p g m", p=P))
            for i in range(3):
                r0, r1 = i * 128, min(OS, i * 128 + 128)
                nc.sync.dma_start(wo_sb[i][:r1 - r0, :], wo16[r0:r1, :])
            nc.vector.memset(wo_sb[2][64:128, :].bitcast(U16), 0)
            nc.sync.dma_start(gb_sb[:], gb16[:, :])
            nc.sync.dma_start(gv_sb[:], gv[:, :])
            for h in range(HG):
                z0 = 64 if h % 2 == 0 else 0
                nc.vector.memset(kTz[h][z0:z0 + 64, :].bitcast(U16), 0)
            nc.vector.memset(qT[2][64:128, :].bitcast(U16), 0)
            nc.vector.memset(mT[2][64:128, :].bitcast(U16), 0)
            for br in range(2):
                for t in range(8):
                    va = va16[br][t][:].rearrange("p (g c) -> p g c", c=D + 1)
                    nc.vector.memset(va[:, :, D:D + 1], 1.0)

            def body(_iv=None):
                CW = 512  # xt chunk width

                # ---------- projections ----------
                def proj_qk(region, sch, copy_eng):
                    col0 = region * SQ + sch * CW
                    xta = work.tile([P, NCHUNK * CW], BF16, tag="xta",
                                    name="xta", bufs=3)
                    nc.sync.dma_start(
                        xta[:].rearrange("p (g c) -> p g c", c=CW),
                        xt16[0:C, col0:col0 + CW].rearrange(
                            "(g p) c -> p g c", p=P))
                    w_sb = wq_sb if region == 0 else wk_sb
                    for blk in range(3):
                        m0 = blk * 128
                        m1 = min(OS, m0 + 128)
                        m = m1 - m0
                        ps = ps_l.tile([P, 1024], F32, tag="L",
                                       name="pqk")[:m, :CW]
                        for cc in range(NCHUNK):
                            nc.tensor.matmul(
                                ps, w_sb[:, cc * OS + m0:cc * OS + m1],
                                xta[:, cc * CW:(cc + 1) * CW],
                                start=(cc == 0), stop=(cc == NCHUNK - 1))
                        cols = slice(sch * CW, (sch + 1) * CW)
                        if region == 0:
                            dst = qT[blk][:m, cols]
                            if copy_eng == "act":
                                nc.scalar.copy(dst, ps)
                            else:
                                nc.vector.tensor_copy(dst, ps)
                        else:
                            h0 = blk * 2
                            gcols = slice((sch - 2) * CW, (sch - 1) * CW)
                            if sch >= 2:  # face: fold gate
                                nc.vector.tensor_tensor(
                                    kTz[h0][0:64, cols], ps[0:64, :],
                                    gb_sb[0:64, gcols], MULT)
                                if m > 64:
                                    nc.vector.tensor_tensor(
                                        kTz[h0 + 1][64:128, cols],
                                        ps[64:128, :],
                                        gb_sb[64:128, gcols], MULT)
                            else:
                                nc.scalar.copy(kTz[h0][0:64, cols],
                                               ps[0:64, :])
                                if m > 64:
                                    nc.scalar.copy(
                                        kTz[h0 + 1][64:128, cols],
                                        ps[64:128, :])
                    return xta

                def proj_v(sch, xta):
                    for st in range(CW // P):
                        t_idx = sch * 4 + st  # global sk tile 0..15
                        br, tt = t_idx // 8, t_idx % 8
                        ps = ps_l.tile([P, 1024], F32, tag="L",
                                       name="pv")[:, :OS]
                        for cc in range(NCHUNK):
                            nc.tensor.matmul(
                                ps, xta[:, cc * CW + st * P:
                                        cc * CW + (st + 1) * P],
                                wv_sb[:, cc * OS:(cc + 1) * OS],
                                start=(cc == 0), stop=(cc == NCHUNK - 1))
                        dst = va16[br][tt][:].rearrange(
                            "p (g c) -> p g c", c=D + 1)[:, :, 0:D]
                        src = ps[:, 0:OS].rearrange("p (g c) -> p g c", c=D)
                        if br == 0:
                            if t_idx % 2 == 0:
                                nc.scalar.copy(dst, src)
                            else:
                                nc.vector.tensor_copy(dst, src)
                        else:
                            if t_idx % 2 == 0:
                                nc.scalar.activation(
                                    dst, src,
                                    mybir.ActivationFunctionType.Copy,
                                    scale=gv_sb[:, t_idx - 8:t_idx - 7])
                            else:
                                nc.vector.tensor_scalar_mul(
                                    dst, src, gv_sb[:, t_idx - 8:t_idx - 7])

                # k/v region first, then the first half of q
                for sch in range(4):
                    xta = proj_qk(1, sch, "act")
                    proj_v(sch, xta)
                for sch in range(2):
                    proj_qk(0, sch, "act")

                # ---------- Wo (emitted on demand, interleaved) ----------
                def wo_chunk(ot, sqh, copy_eng):
                    o0 = ot * P
                    pss = ps_l.tile([P, 1024], F32, tag="L", name="pwo")
                    for cc in range(3):
                        for n2 in range(2):
                            c0 = sqh * 1024 + n2 * 512
                            nc.tensor.matmul(
                                pss[:, n2 * 512:(n2 + 1) * 512],
                                wo_sb[cc][:, o0:o0 + P],
                                mT[cc][:, c0:c0 + 512],
                                start=(cc == 0), stop=(cc == 2))
                    ob = outsb.tile([P, 1024], BF16, tag="ob", name="ob")
                    if copy_eng == "act":
                        nc.scalar.copy(ob[:], pss[:])
                    else:
                        nc.vector.tensor_copy(ob[:], pss[:])
                    nc.sync.dma_start(
                        outp[o0:o0 + P, sqh * 1024:(sqh + 1) * 1024], ob[:])

                # ---------- attention ----------
                # round = (sqh, h); 16 slots (sk, br)
                def make_tail(i, h, q0, avsb, rg, rgb, rb, prod):
                    def st(ops):
                        def run():
                            for op in ops:
                                op()
                        return run
                    ops = []
                    ops.append(st([
                        lambda: nc.sync.dma_start(rg[0:1, :],
                                                  avsb[64:65, :])]))
                    ops.append(st([
                        lambda: nc.vector.reciprocal(rgb[0:1, :], rg[0:1, :])]))
                    # broadcast partition 0 -> 64 via 32-lane DVE shuffles
                    ops.append(st([
                        lambda: nc.vector.stream_shuffle(
                            rb[0][0:32, :], rgb[0:32, 0:1024], [0] * 32),
                        lambda: nc.vector.stream_shuffle(
                            rb[1][0:32, :], rgb[0:32, 1024:2048], [0] * 32)]))
                    ops.append(st([
                        lambda: nc.vector.stream_shuffle(
                            rb[0][32:64, :], rb[0][0:32, :], [0] * 32),
                        lambda: nc.vector.stream_shuffle(
                            rb[1][32:64, :], rb[1][0:32, :], [0] * 32)]))
                    ops.append(st([
                        lambda: nc.vector.tensor_tensor(
                            prod[0][0:64, :], avsb[0:64, 0:1024],
                            rb[0][0:64, :], MULT),
                        lambda: nc.vector.tensor_tensor(
                            prod[1][0:64, :], avsb[0:64, 1024:2048],
                            rb[1][0:64, :], MULT)]))
                    if h % 2 == 0:
                        ops.append(st([
                            lambda: nc.vector.tensor_add(
                                mT[i][0:64, q0:q0 + 1024],
                                prod[0][0:64, :], prod[1][0:64, :])]))
                    else:
                        ops.append(st([
                            lambda: nc.vector.tensor_add(
                                prod[0][0:64, :], prod[0][0:64, :],
                                prod[1][0:64, :]),
                            lambda: nc.sync.dma_start(
                                mT[i][64:128, q0:q0 + 1024],
                                prod[0][0:64, :])]))
                    return ops

                TAIL_SLOT = {0: 0, 2: 1, 4: 2, 5: 3, 9: 4, 12: 5}
                AV_LAG = 2
                pending = []
                rounds = [] if abl_skip_attn else [
                    (sqh, h) for sqh in range(2) for h in range(HG)]
                for ri, (sqh, h) in enumerate(rounds):
                    i = h // 2
                    q0 = sqh * 1024
                    avps = [ps_av.tile([65, 512], F32, tag="av",
                                       name=f"av{j}") for j in range(4)]
                    avsb = work.tile([65, 2048], BF16, tag="avsb",
                                     name="avsb")
                    av_q = []
                    slot = 0
                    for br in range(2):
                        for sk in range(8):
                            L = ps_l.tile([P, 1024], F32, tag="L", name="L")
                            kcol = br * S + sk * P
                            for n2 in range(2):
                                nc.tensor.matmul(
                                    L[:, n2 * 512:(n2 + 1) * 512],
                                    kTz[h][:, kcol:kcol + P],
                                    qT[i][:, q0 + n2 * 512:
                                          q0 + (n2 + 1) * 512],
                                    start=True, stop=True)
                            pt16 = ptp.tile([P, 1024], BF16, tag="pt",
                                            name="pt")
                            nc.scalar.activation(pt16[:], L[:], EXP,
                                                 scale=0.125)

                            def av_op(sk=sk, br=br, pt16=pt16):
                                va = va16[br][sk][:, h * (D + 1):
                                                  h * (D + 1) + D + 1]
                                for n2 in range(2):
                                    nc.tensor.matmul(
                                        avps[br * 2 + n2][:, :], va,
                                        pt16[:, n2 * 512:(n2 + 1) * 512],
                                        start=(sk == 0), stop=(sk == 7))
                            av_q.append(av_op)
                            if slot >= AV_LAG:
                                av_q[slot - AV_LAG]()
                            if slot in TAIL_SLOT and \
                                    TAIL_SLOT[slot] < len(pending):
                                pending[TAIL_SLOT[slot]]()
                            slot += 1
                            if slot == 10:
                                for j in range(2):
                                    nc.vector.tensor_copy(
                                        avsb[:, j * 512:(j + 1) * 512],
                                        avps[j][0:65, :])
                    for op in av_q[16 - AV_LAG:]:
                        op()
                    for stg in pending[len(TAIL_SLOT):]:
                        stg()
                    # interleave remaining q projection with early rounds,
                    # and sqh=0's Wo chunks with rounds 5..9
                    if ri < 2:
                        proj_qk(0, 2 + ri, "dve")
                    if not abl_skip_wo:
                        if ri >= 5:
                            wo_chunk(2 * (ri - 5), 0, "dve")
                            wo_chunk(2 * (ri - 5) + 1, 0, "dve")
                    for j in range(2, 4):
                        nc.vector.tensor_copy(
                            avsb[:, j * 512:(j + 1) * 512], avps[j][0:65, :])
                    if abl_no_tail:
                        pending = []
                        continue
                    rg = work.tile([1, 2048], BF16, tag="rg", name="rg")
                    rgb = work.tile([32, 2048], BF16, tag="rgb", name="rgb")
                    rb = [work.tile([64, 1024], BF16, tag=f"rb{br}",
                                    name=f"rb{br}") for br in range(2)]
                    prod = [work.tile([64, 1024], BF16, tag=f"pr{br}",
                                      name=f"pr{br}") for br in range(2)]
                    pending = make_tail(i, h, q0, avsb, rg, rgb, rb, prod)
                for stg in pending:
                    stg()
                if abl_skip_attn:
                    for sch in range(2, 4):
                        proj_qk(0, sch, "dve")

                # ---------- Wo sq-half 1 (and all of it if attn skipped) ----
                for ot in range(0 if abl_skip_wo else NCHUNK):
                    if abl_skip_attn:
                        wo_chunk(ot, 0, "act" if ot % 2 else "dve")
                    wo_chunk(ot, 1, "act" if ot % 2 else "dve")

            if iters > 1:
                with tc.For_i(0, iters, 1):
                    body()
            else:
                body()

    nc.compile()
    return nc


_NC_CACHE = {}


def _get_nc(iters: int = 1):
    if iters not in _NC_CACHE:
        _NC_CACHE[iters] = build_nc(iters)
    return _NC_CACHE[iters]


def make_in_maps(hidden_states, mask_ref, Wq, Wk, Wv, Wo):
    np16 = mybir.dt.np(BF16)
    hsT = np.ascontiguousarray(
        np.asarray(hidden_states, dtype=np.float32).transpose(0, 2, 1))
    mask = np.asarray(mask_ref, dtype=np.float32)
    Wq = np.asarray(Wq, dtype=np.float32)
    Wk = np.asarray(Wk, dtype=np.float32)
    Wv = np.asarray(Wv, dtype=np.float32)
    Wo = np.asarray(Wo, dtype=np.float32)
    in_maps = []
    for b in range(B):
        xt_b = np.concatenate(
            [hsT[2 * b], hsT[2 * b + 1], hsT[b], hsT[2 + b]],
            axis=1).astype(np16)
        gate = mask[b, :, 0]
        gcol = np.ascontiguousarray(gate.reshape(8, P).T)
        gbc = np.ascontiguousarray(
            np.broadcast_to(gate[None, :], (P, S))).astype(np16)
        for g in range(G):
            osl = slice(g * OS, (g + 1) * OS)
            in_maps.append({
                "xt16": xt_b,
                "wq16": np.ascontiguousarray(Wq[osl, :].T).astype(np16),
                "wk16": np.ascontiguousarray(Wk[osl, :].T).astype(np16),
                "wv16": np.ascontiguousarray(Wv[osl, :].T).astype(np16),
                "wo16": np.ascontiguousarray(Wo[:, osl].T).astype(np16),
                "gb16": gbc,
                "gv": gcol,
            })
    return in_maps


def kernel(hidden_states, mask_ref, Wq, Wk, Wv, Wo, bo, heads):
    assert int(heads) == H
    nc = _get_nc(1)
    in_maps = make_in_maps(hidden_states, mask_ref, Wq, Wk, Wv, Wo)
    res = run_bass_kernel_spmd(nc, in_maps, core_ids=list(range(8)))
    bo = np.asarray(bo, dtype=np.float32)
    out = np.empty((B, SQ, C), dtype=np.float32)
    for b in range(B):
        acc = res.results[b * G]["outp"].astype(np.float32)
        for g in range(1, G):
            acc += res.results[b * G + g]["outp"].astype(np.float32)
        out[b] = acc.T + bo
    return out


# revision 46
# speedup vs baseline: 1.8538x; 1.8538x over previous
"""Branched attention processor (SDXL-like) on 8 Trainium2 NeuronCores.

Sharding: 2-way data-parallel over the half-batch dim x 4-way tensor-parallel
over heads (5 heads = 320 features per core). Each core computes a partial
out^T = Wo[:, c_slice] @ merged[c_slice, :]; the host sums the 4 partials per
half-batch and adds the bias.

v3: bf16 numerics end to end (fp8 injects >=4% per-weight noise which the
softmax passes through at full strength -- measured, not theoretical).
  - xt / weights / q / k / v / pt / merged all bf16; PSUM accumulate f32.
  - k stored per head zero-padded to 128 rows (full-rate contraction);
    face-branch gate folded into k/v data so exp needs no per-partition
    scale and both branches share one activation.
  - exp runs on Act only (nothing else can do exp); everything else is kept
    off Act during attention so its 16 x 1.04us per round sets the pace.
  - Tail: PSUM accumulators drained to SBUF bf16 (DVE), sums row DMA'd to
    partition 0, one reciprocal (DVE), gpsimd partition_broadcast (Pool),
    bf16 normalize mults (Pool) + merge add (DVE); software-pipelined into
    the next round's slots. AV matmuls are emitted with a 4-slot lag so the
    in-order PE queue never head-blocks on the previous round's PSUM WAR.
  - Wo for sq-half 0 is interleaved into attention rounds 5..9 (mT cols
    0:1024 are complete after round 4); only half 1 remains serial.
"""

import os as _os

import numpy as np

import concourse.bass as bass
import concourse.tile as tile
import concourse.mybir as mybir
from concourse import bacc
from concourse.bass_utils import run_bass_kernel_spmd

# Problem shapes (hardcoded per contract)
B2, S, C = 4, 1024, 1280
B = B2 // 2           # 2 half-batches
H = 20                # heads
D = C // H            # 64
G = 4                 # head groups (tensor-parallel)
HG = H // G           # 5 heads per core
OS = HG * D           # 320 features per core
SQ = 2 * S            # 2048 queries per half-batch
P = 128
NCHUNK = C // P       # 10 c-chunks of 128

F32 = mybir.dt.float32
BF16 = mybir.dt.bfloat16
U16 = mybir.dt.uint16
EXP = mybir.ActivationFunctionType.Exp
MULT = mybir.AluOpType.mult


def build_nc(iters: int = 1):
    nc = bacc.Bacc("TRN2", target_bir_lowering=False, debug=False, num_devices=8)

    xt16 = nc.dram_tensor("xt16", [C, 2 * SQ], BF16, kind="ExternalInput")
    wq16 = nc.dram_tensor("wq16", [C, OS], BF16, kind="ExternalInput")
    wk16 = nc.dram_tensor("wk16", [C, OS], BF16, kind="ExternalInput")
    wv16 = nc.dram_tensor("wv16", [C, OS], BF16, kind="ExternalInput")
    wo16 = nc.dram_tensor("wo16", [OS, C], BF16, kind="ExternalInput")
    gb16 = nc.dram_tensor("gb16", [P, S], BF16, kind="ExternalInput")
    gv = nc.dram_tensor("gv", [P, 8], F32, kind="ExternalInput")
    outp = nc.dram_tensor("outp", [C, SQ], BF16, kind="ExternalOutput")

    abl_skip_attn = bool(_os.environ.get("KSKIP_ATTN"))
    abl_skip_wo = bool(_os.environ.get("KSKIP_WO"))
    abl_no_tail = bool(_os.environ.get("KATT_NOTAIL"))

    with tile.TileContext(nc) as tc, nc.allow_low_precision(
            reason="bf16 kernel by design"):
        with (
            tc.tile_pool(name="persist", bufs=1) as persist,
            tc.tile_pool(name="work", bufs=3) as work,
            tc.tile_pool(name="ptp", bufs=8) as ptp,
            tc.tile_pool(name="outsb", bufs=2) as outsb,
            tc.tile_pool(name="ps_l", bufs=2, space="PSUM") as ps_l,
            tc.tile_pool(name="ps_av", bufs=4, space="PSUM") as ps_av,
        ):
            # ---- persistent tiles ----
            # q^T per unit (pairs of heads); unit 2 rows 64:128 unused
            qT = [persist.tile([P, SQ], BF16, tag=f"qT{i}", name=f"qT{i}")
                  for i in range(3)]
            # k^T per head, zero-padded to 128 rows: head h lives at rows
            # (h%2)*64, the other 64 rows stay zero (full 128-contraction).
            # cols 0:1024 noise, 1024:2048 ref (gate folded in).
            kTz = [persist.tile([P, SQ], BF16, tag=f"kTz{h}", name=f"kTz{h}")
                   for h in range(HG)]
            # v_aug per (branch, sk tile): [128, 5, 65]: 64 v cols + ones
            va16 = [[persist.tile([P, HG * (D + 1)], BF16, tag=f"va{br}_{t}",
                                  name=f"va{br}_{t}")
                     for t in range(8)] for br in range(2)]
            # merged^T bf16 per unit; unit 2 rows 64:128 zeroed
            mT = [persist.tile([P, SQ], BF16, tag=f"mT{i}", name=f"mT{i}")
                  for i in range(3)]
            wq_sb = persist.tile([P, NCHUNK * OS], BF16, tag="wq")
            wk_sb = persist.tile([P, NCHUNK * OS], BF16, tag="wk")
            wv_sb = persist.tile([P, NCHUNK * OS], BF16, tag="wv")
            wo_sb = [persist.tile([P, C], BF16, tag=f"wo{i}", name=f"wo{i}")
                     for i in range(3)]
            gb_sb = persist.tile([P, S], BF16, tag="gb")
            gv_sb = persist.tile([P, 8], F32, tag="gvs")

            for w_sb, w_dram in ((wq_sb, wq16), (wk_sb, wk16), (wv_sb, wv16)):
                nc.sync.dma_start(
                    w_sb[:].rearrange("p (g m) -> p g m", m=OS),
                    w_dram[0:C, :].rearrange("(g p) m -> p g m", p=P))
            for i in range(3):
                r0, r1 = i * 128, min(OS, i * 128 + 128)
                nc.sync.dma_start(wo_sb[i][:r1 - r0, :], wo16[r0:r1, :])
            nc.vector.memset(wo_sb[2][64:128, :].bitcast(U16), 0)
            nc.sync.dma_start(gb_sb[:], gb16[:, :])
            nc.sync.dma_start(gv_sb[:], gv[:, :])
            for h in range(HG):
                z0 = 64 if h % 2 == 0 else 0
                nc.vector.memset(kTz[h][z0:z0 + 64, :].bitcast(U16), 0)
            nc.vector.memset(qT[2][64:128, :].bitcast(U16), 0)
            nc.vector.memset(mT[2][64:128, :].bitcast(U16), 0)
            for br in range(2):
                for t in range(8):
                    va = va16[br][t][:].rearrange("p (g c) -> p g c", c=D + 1)
                    nc.vector.memset(va[:, :, D:D + 1], 1.0)

            def body(_iv=None):
                CW = 512  # xt chunk width

                # ---------- projections ----------
                def proj_qk(region, sch, copy_eng):
                    col0 = region * SQ + sch * CW
                    xta = work.tile([P, NCHUNK * CW], BF16, tag="xta",
                                    name="xta", bufs=4)
                    nc.sync.dma_start(
                        xta[:].rearrange("p (g c) -> p g c", c=CW),
                        xt16[0:C, col0:col0 + CW].rearrange(
                            "(g p) c -> p g c", p=P))
                    w_sb = wq_sb if region == 0 else wk_sb
                    for blk in range(3):
                        m0 = blk * 128
                        m1 = min(OS, m0 + 128)
                        m = m1 - m0
                        ps = ps_l.tile([P, 1024], F32, tag="L",
                                       name="pqk")[:m, :CW]
                        for cc in range(NCHUNK):
                            nc.tensor.matmul(
                                ps, w_sb[:, cc * OS + m0:cc * OS + m1],
                                xta[:, cc * CW:(cc + 1) * CW],
                                start=(cc == 0), stop=(cc == NCHUNK - 1))
                        cols = slice(sch * CW, (sch + 1) * CW)
                        if region == 0:
                            dst = qT[blk][:m, cols]
                            if copy_eng == "act":
                                nc.scalar.copy(dst, ps)
                            else:
                                nc.vector.tensor_copy(dst, ps)
                        else:
                            h0 = blk * 2
                            gcols = slice((sch - 2) * CW, (sch - 1) * CW)
                            if sch >= 2:  # face: fold gate
                                nc.vector.tensor_tensor(
                                    kTz[h0][0:64, cols], ps[0:64, :],
                                    gb_sb[0:64, gcols], MULT)
                                if m > 64:
                                    nc.vector.tensor_tensor(
                                        kTz[h0 + 1][64:128, cols],
                                        ps[64:128, :],
                                        gb_sb[64:128, gcols], MULT)
                            else:
                                nc.scalar.copy(kTz[h0][0:64, cols],
                                               ps[0:64, :])
                                if m > 64:
                                    nc.scalar.copy(
                                        kTz[h0 + 1][64:128, cols],
                                        ps[64:128, :])
                    return xta

                def proj_v(sch, xta):
                    for st in range(CW // P):
                        t_idx = sch * 4 + st  # global sk tile 0..15
                        br, tt = t_idx // 8, t_idx % 8
                        ps = ps_l.tile([P, 1024], F32, tag="L",
                                       name="pv")[:, :OS]
                        for cc in range(NCHUNK):
                            nc.tensor.matmul(
                                ps, xta[:, cc * CW + st * P:
                                        cc * CW + (st + 1) * P],
                                wv_sb[:, cc * OS:(cc + 1) * OS],
                                start=(cc == 0), stop=(cc == NCHUNK - 1))
                        dst = va16[br][tt][:].rearrange(
                            "p (g c) -> p g c", c=D + 1)[:, :, 0:D]
                        src = ps[:, 0:OS].rearrange("p (g c) -> p g c", c=D)
                        if br == 0:
                            if t_idx % 2 == 0:
                                nc.scalar.copy(dst, src)
                            else:
                                nc.vector.tensor_copy(dst, src)
                        else:
                            if t_idx % 2 == 0:
                                nc.scalar.activation(
                                    dst, src,
                                    mybir.ActivationFunctionType.Copy,
                                    scale=gv_sb[:, t_idx - 8:t_idx - 7])
                            else:
                                nc.vector.tensor_scalar_mul(
                                    dst, src, gv_sb[:, t_idx - 8:t_idx - 7])

                # k/v region first, then the first half of q
                for sch in range(4):
                    xta = proj_qk(1, sch, "act")
                    proj_v(sch, xta)
                for sch in range(2):
                    proj_qk(0, sch, "act")

                # ---------- Wo (emitted on demand, interleaved) ----------
                def wo_chunk(ot, sqh, copy_eng):
                    o0 = ot * P
                    pss = ps_l.tile([P, 1024], F32, tag="L", name="pwo")
                    for cc in range(3):
                        for n2 in range(2):
                            c0 = sqh * 1024 + n2 * 512
                            nc.tensor.matmul(
                                pss[:, n2 * 512:(n2 + 1) * 512],
                                wo_sb[cc][:, o0:o0 + P],
                                mT[cc][:, c0:c0 + 512],
                                start=(cc == 0), stop=(cc == 2))
                    ob = outsb.tile([P, 1024], BF16, tag="ob", name="ob")
                    if copy_eng == "act":
                        nc.scalar.copy(ob[:], pss[:])
                    else:
                        nc.vector.tensor_copy(ob[:], pss[:])
                    nc.sync.dma_start(
                        outp[o0:o0 + P, sqh * 1024:(sqh + 1) * 1024], ob[:])

                # ---------- attention ----------
                # round = (sqh, h); 16 slots (sk, br)
                def make_tail(i, h, q0, avsb, rg, rgb, rb, prod):
                    def st(ops):
                        def run():
                            for op in ops:
                                op()
                        return run
                    ops = []
                    ops.append(st([
                        lambda: nc.sync.dma_start(rg[0:1, :],
                                                  avsb[64:65, :])]))
                    ops.append(st([
                        lambda: nc.vector.reciprocal(rgb[0:1, :], rg[0:1, :])]))
                    # broadcast partition 0 -> 64 via 32-lane DVE shuffles
                    ops.append(st([
                        lambda: nc.vector.stream_shuffle(
                            rb[0][0:32, :], rgb[0:32, 0:1024], [0] * 32),
                        lambda: nc.vector.stream_shuffle(
                            rb[1][0:32, :], rgb[0:32, 1024:2048], [0] * 32)]))
                    ops.append(st([
                        lambda: nc.vector.stream_shuffle(
                            rb[0][32:64, :], rb[0][0:32, :], [0] * 32),
                        lambda: nc.vector.stream_shuffle(
                            rb[1][32:64, :], rb[1][0:32, :], [0] * 32)]))
                    ops.append(st([
                        lambda: nc.vector.tensor_tensor(
                            prod[0][0:64, :], avsb[0:64, 0:1024],
                            rb[0][0:64, :], MULT),
                        lambda: nc.vector.tensor_tensor(
                            prod[1][0:64, :], avsb[0:64, 1024:2048],
                            rb[1][0:64, :], MULT)]))
                    if h % 2 == 0:
                        ops.append(st([
                            lambda: nc.vector.tensor_add(
                                mT[i][0:64, q0:q0 + 1024],
                                prod[0][0:64, :], prod[1][0:64, :])]))
                    else:
                        ops.append(st([
                            lambda: nc.vector.tensor_add(
                                prod[0][0:64, :], prod[0][0:64, :],
                                prod[1][0:64, :]),
                            lambda: nc.sync.dma_start(
                                mT[i][64:128, q0:q0 + 1024],
                                prod[0][0:64, :])]))
                    return ops

                TAIL_SLOT = {0: 0, 2: 1, 4: 2, 5: 3, 9: 4, 12: 5}
                AV_LAG = 4
                pending = []
                rounds = [] if abl_skip_attn else [
                    (sqh, h) for sqh in range(2) for h in range(HG)]
                for ri, (sqh, h) in enumerate(rounds):
                    i = h // 2
                    q0 = sqh * 1024
                    avps = [ps_av.tile([65, 512], F32, tag="av",
                                       name=f"av{j}") for j in range(4)]
                    av_q = []
                    slot = 0
                    for sk in range(8):
                        for br in range(2):
                            L = ps_l.tile([P, 1024], F32, tag="L", name="L")
                            kcol = br * S + sk * P
                            for n2 in range(2):
                                nc.tensor.matmul(
                                    L[:, n2 * 512:(n2 + 1) * 512],
                                    kTz[h][:, kcol:kcol + P],
                                    qT[i][:, q0 + n2 * 512:
                                          q0 + (n2 + 1) * 512],
                                    start=True, stop=True)
                            pt16 = ptp.tile([P, 1024], BF16, tag="pt",
                                            name="pt")
                            nc.scalar.activation(pt16[:], L[:], EXP,
                                                 scale=0.125)

                            def av_op(sk=sk, br=br, pt16=pt16):
                                va = va16[br][sk][:, h * (D + 1):
                                                  h * (D + 1) + D + 1]
                                for n2 in range(2):
                                    nc.tensor.matmul(
                                        avps[br * 2 + n2][:, :], va,
                                        pt16[:, n2 * 512:(n2 + 1) * 512],
                                        start=(sk == 0), stop=(sk == 7))
                            av_q.append(av_op)
                            if slot >= AV_LAG:
                                av_q[slot - AV_LAG]()
                            if slot in TAIL_SLOT and \
                                    TAIL_SLOT[slot] < len(pending):
                                pending[TAIL_SLOT[slot]]()
                            slot += 1
                    for op in av_q[16 - AV_LAG:]:
                        op()
                    for stg in pending[len(TAIL_SLOT):]:
                        stg()
                    # interleave remaining q projection with early rounds,
                    # and sqh=0's Wo chunks with rounds 5..9
                    if ri < 2:
                        proj_qk(0, 2 + ri, "dve")
                    if not abl_skip_wo:
                        if ri >= 5:
                            wo_chunk(2 * (ri - 5), 0, "dve")
                            wo_chunk(2 * (ri - 5) + 1, 0, "dve")
                    avsb = work.tile([65, 2048], BF16, tag="avsb",
                                     name="avsb")
                    for j in range(4):
                        nc.vector.tensor_copy(
                            avsb[:, j * 512:(j + 1) * 512], avps[j][0:65, :])
                    if abl_no_tail:
                        pending = []
                        continue
                    rg = work.tile([1, 2048], BF16, tag="rg", name="rg")
                    rgb = work.tile([32, 2048], BF16, tag="rgb", name="rgb")
                    rb = [work.tile([64, 1024], BF16, tag=f"rb{br}",
                                    name=f"rb{br}") for br in range(2)]
                    prod = [work.tile([64, 1024], BF16, tag=f"pr{br}",
                                      name=f"pr{br}") for br in range(2)]
                    pending = make_tail(i, h, q0, avsb, rg, rgb, rb, prod)
                for stg in pending:
                    stg()
                if abl_skip_attn:
                    for sch in range(2, 4):
                        proj_qk(0, sch, "dve")

                # ---------- Wo sq-half 1 (and all of it if attn skipped) ----
                for ot in range(0 if abl_skip_wo else NCHUNK):
                    if abl_skip_attn:
                        wo_chunk(ot, 0, "act" if ot % 2 else "dve")
                    wo_chunk(ot, 1, "act" if ot % 2 else "dve")

            if iters > 1:
                with tc.For_i(0, iters, 1):
                    body()
            else:
                body()

    nc.compile()
    return nc


_NC_CACHE = {}


def _get_nc(iters: int = 1):
    if iters not in _NC_CACHE:
        _NC_CACHE[iters] = build_nc(iters)
    return _NC_CACHE[iters]


def make_in_maps(hidden_states, mask_ref, Wq, Wk, Wv, Wo):
    np16 = mybir.dt.np(BF16)
    hsT = np.ascontiguousarray(
        np.asarray(hidden_states, dtype=np.float32).transpose(0, 2, 1))
    mask = np.asarray(mask_ref, dtype=np.float32)
    Wq = np.asarray(Wq, dtype=np.float32)
    Wk = np.asarray(Wk, dtype=np.float32)
    Wv = np.asarray(Wv, dtype=np.float32)
    Wo = np.asarray(Wo, dtype=np.float32)
    in_maps = []
    for b in range(B):
        xt_b = np.concatenate(
            [hsT[2 * b], hsT[2 * b + 1], hsT[b], hsT[2 + b]],
            axis=1).astype(np16)
        gate = mask[b, :, 0]
        gcol = np.ascontiguousarray(gate.reshape(8, P).T)
        gbc = np.ascontiguousarray(
            np.broadcast_to(gate[None, :], (P, S))).astype(np16)
        for g in range(G):
            osl = slice(g * OS, (g + 1) * OS)
            in_maps.append({
                "xt16": xt_b,
                "wq16": np.ascontiguousarray(Wq[osl, :].T).astype(np16),
                "wk16": np.ascontiguousarray(Wk[osl, :].T).astype(np16),
                "wv16": np.ascontiguousarray(Wv[osl, :].T).astype(np16),
                "wo16": np.ascontiguousarray(Wo[:, osl].T).astype(np16),
                "gb16": gbc,
                "gv": gcol,
            })
    return in_maps


def kernel(hidden_states, mask_ref, Wq, Wk, Wv, Wo, bo, heads):
    assert int(heads) == H
    nc = _get_nc(1)
    in_maps = make_in_maps(hidden_states, mask_ref, Wq, Wk, Wv, Wo)
    res = run_bass_kernel_spmd(nc, in_maps, core_ids=list(range(8)))
    bo = np.asarray(bo, dtype=np.float32)
    out = np.empty((B, SQ, C), dtype=np.float32)
    for b in range(B):
        acc = res.results[b * G]["outp"].astype(np.float32)
        for g in range(1, G):
            acc += res.results[b * G + g]["outp"].astype(np.float32)
        out[b] = acc.T + bo
    return out
